# revision 12
# baseline (speedup 1.0000x reference)
"""Trainium2 Bass kernel for nn_ConvGraphSelfLoop.

out = where(any(adj>=0, axes -1,-2), relu(features @ W + b), features)

Strategy (device does the GEMM, host does layout + select):
  - A vertex is "valid" iff any adjacency entry >= 0. Invalid vertices
    pass their input features through untouched — the host writes those
    directly from the fp32 input, so the device only transforms valid
    vertices (~75% of 65536).
  - Host compacts the valid vertices, casts to fp16, transposes to
    xT [F, n] and splits them evenly across 8 cores (capacity 6656
    tokens/core = 13 blocks of 512; valid count 49152 +- 111, so 6656
    per core is a +37 sigma bound. Any overflow beyond capacity is
    computed on the host — correctness never depends on the bound).
  - Device computes outT = relu(W^T @ xT + b) in transposed space:
      * W [F, U] already has the contraction dim on partitions, so W
        chunks are the stationary operand — NO PE transposes at all.
      * bias lands on partitions (u-chunks), so it rides the ACT
        eviction (activation bias operand) — no bias matmuls.
      * fp16 operands: 1 cyc/row PE rate (same as bf16), half the DMA.
  - Per core: 13 token-blocks of 512, grouped in superblocks of 2048
    so each stationary W chunk serves 4 consecutive matmuls.
    PE work = 832 matmuls x 512 rows ~= 178 us; ACT evicts psum with
    relu+bias; DMA in/out ~28 MB fully overlapped.
"""
import numpy as np
import concourse.bass as bass
import concourse.bacc as bacc
import concourse.mybir as mybir
import concourse.tile as tile
from concourse.bass_utils import run_bass_kernel_spmd

B, V, E, NN = 4, 16384, 4, 32
F, U = 1024, 1024
NCORES = 8
P = 128
BLK = 512                    # tokens per psum bank / matmul free dim
NBLK = 13                    # token-blocks per core (capacity 6656)
CAP = NBLK * BLK             # 6656 tokens per core
SUPERS = [(0, 512), (512, 1024), (1536, 2048), (3584, 2048), (5632, 1024)]
NWARM = 14                   # PE prewarm matmuls issued during DMA fill
CF = F // P                  # 8 contraction chunks
CU = U // P                  # 8 output-partition chunks

f32 = mybir.dt.float32
f16 = mybir.dt.float16
AF = mybir.ActivationFunctionType


def _build():
    nc = bacc.Bacc("TRN2", target_bir_lowering=False, debug=False,
                   num_devices=NCORES)
    xt_d = nc.dram_tensor("xt", [F, CAP], f16, kind="ExternalInput")
    w_d = nc.dram_tensor("weight", [F, U], f16, kind="ExternalInput")
    bias_d = nc.dram_tensor("bias", [P, CU], f32, kind="ExternalInput")
    out_d = nc.dram_tensor("outT", [U, CAP], f16, kind="ExternalOutput")
    scratch_d = nc.dram_tensor("scratch", [P, 1], f16, kind="ExternalOutput")

    with tile.TileContext(nc) as tc:
        with tc.tile_pool(name="const", bufs=1) as const, \
             tc.tile_pool(name="xp", bufs=2) as xp, \
             tc.tile_pool(name="op", bufs=3) as op, \
             tc.tile_pool(name="psp", bufs=2, space="PSUM") as psp:

            # ---- PE prewarm: keep the PE busy from t=0 so the HAM
            # clock-gate is at 8/8 when the real matmuls start; runs
            # while the weight/x DMA fill is in flight. Consumed via a
            # scratch store so DCE keeps it.
            # Consumers stay OFF the critical queues: DVE evicts the warm
            # psum (ACT does the real evictions) and the scratch store
            # goes via the gpsimd DMA queue (sync queue carries the input
            # fill — a blocked sync dma_start would stall the whole fill).
            warm_in = const.tile([P, BLK], f16)
            nc.gpsimd.memset(warm_in[:], 0.0)
            wps = psp.tile([P, BLK], f32, tag="ps")
            for i in range(NWARM):
                nc.tensor.matmul(wps[:], warm_in[:, 0:P], warm_in[:],
                                 start=(i == 0), stop=(i == NWARM - 1))
            warm_out = op.tile([P, 1], f16, tag="warm")
            nc.vector.tensor_copy(out=warm_out[:], in_=wps[:, 0:1])
            nc.gpsimd.dma_start(scratch_d.ap()[:, :], warm_out[:])

            # ---- resident constants: W chunks + bias ----
            # w_sb[:, f*U + j] = W[f*P + p, j]  (slab f = W rows f*P..)
            # Issued from the (otherwise idle at startup) Vector/Scalar
            # queues so the fill builds DMA depth in parallel with the
            # xs0 loads on the Sync queue — one engine alone issues a
            # dma_start only every ~650ns and underfeeds the HW queues.
            w_sb = const.tile([P, CF * U], f16)
            for f in range(CF):
                eng = nc.gpsimd if f % 2 == 0 else nc.scalar
                eng.dma_start(w_sb[:, f * U:(f + 1) * U],
                              w_d.ap()[f * P:(f + 1) * P, :])
            bias_sb = const.tile([P, CU], f32)
            nc.gpsimd.dma_start(bias_sb[:], bias_d.ap())

            for off, W in SUPERS:
                nb = W // BLK
                # xs holds CF f-chunk slabs side by side: chunk f at
                # cols [f*W, (f+1)*W)
                xs = xp.tile([P, CF * W], f16, tag="xs")
                for f in range(CF):
                    nc.sync.dma_start(xs[:, f * W:(f + 1) * W],
                                      xt_d.ap()[f * P:(f + 1) * P,
                                                off:off + W])
                for u in range(CU):
                    ps = psp.tile([P, W], f32, tag="ps")
                    for f in range(CF):
                        lhsT = w_sb[:, f * U + u * P: f * U + (u + 1) * P]
                        for b in range(nb):
                            nc.tensor.matmul(
                                ps[:, b * BLK:(b + 1) * BLK],
                                lhsT,
                                xs[:, f * W + b * BLK: f * W + (b + 1) * BLK],
                                start=(f == 0), stop=(f == CF - 1))
                    r = op.tile([P, W], f16, tag="r")
                    nc.scalar.activation(r[:], ps[:], AF.Relu,
                                         bias=bias_sb[:, u:u + 1])
                    nc.sync.dma_start(out_d.ap()[u * P:(u + 1) * P,
                                                 off:off + W], r[:])

    nc.compile()
    return nc


_nc_cache = None


def _get_nc():
    global _nc_cache
    if _nc_cache is None:
        _nc_cache = _build()
    return _nc_cache


def _preprocess(inputs):
    """Host-side: mask, compaction, fp16 transpose, per-core split."""
    feats2 = np.asarray(inputs["features"], dtype=np.float32).reshape(B * V, F)
    adj2 = np.asarray(inputs["adjacency"]).reshape(B * V, E * NN)
    valid = adj2.max(axis=1) >= 0
    idx = np.flatnonzero(valid)
    dev_idx = idx[:NCORES * CAP]          # device-computed valid tokens
    ovf_idx = idx[NCORES * CAP:]          # host fallback (statistically never)

    w16 = np.ascontiguousarray(inputs["kernel"], dtype=np.float16)
    bias = np.asarray(inputs["bias"], dtype=np.float32).reshape(-1)
    bias_dev = np.ascontiguousarray(bias.reshape(CU, P).T, dtype=np.float32)

    n = dev_idx.size
    counts = [(n + NCORES - 1 - i) // NCORES for i in range(NCORES)]
    starts = np.cumsum([0] + counts)
    in_maps, core_idx = [], []
    for i in range(NCORES):
        ci = dev_idx[starts[i]:starts[i + 1]]
        core_idx.append(ci)
        xti = np.zeros((F, CAP), dtype=np.float16)
        if ci.size:
            xti[:, :ci.size] = feats2[ci].T.astype(np.float16)
        in_maps.append({"xt": xti, "weight": w16, "bias": bias_dev})
    return feats2, valid, core_idx, ovf_idx, in_maps


def _make_in_maps(inputs):
    return _preprocess(inputs)[4]


def kernel(adjacency, features, kernel, bias):
    nc = _get_nc()
    inputs = {"adjacency": adjacency, "features": features,
              "kernel": kernel, "bias": bias}
    feats2, valid, core_idx, ovf_idx, in_maps = _preprocess(inputs)
    res = run_bass_kernel_spmd(nc, in_maps, list(range(NCORES)))

    out = np.empty((B * V, U), dtype=np.float32)
    out[~valid] = feats2[~valid]
    for i in range(NCORES):
        ci = core_idx[i]
        if ci.size:
            oT = res.results[i]["outT"]
            out[ci] = oT[:, :ci.size].T.astype(np.float32)
    if ovf_idx.size:
        w32 = np.asarray(kernel, dtype=np.float32)
        b32 = np.asarray(bias, dtype=np.float32).reshape(-1)
        out[ovf_idx] = np.maximum(feats2[ovf_idx] @ w32 + b32, 0.0)
    return out.reshape(B, V, U)


# revision 13
# speedup vs baseline: 1.1518x; 1.1518x over previous
"""Trainium2 Bass kernel for nn_ConvGraphSelfLoop.

out = where(any(adj>=0, axes -1,-2), relu(features @ W + b), features)

Strategy (device does the GEMM, host does layout + select):
  - A vertex is "valid" iff any adjacency entry >= 0. Invalid vertices
    pass their input features through untouched — the host writes those
    directly from the fp32 input, so the device only transforms valid
    vertices (~75% of 65536).
  - Host compacts the valid vertices, casts to fp16, transposes to
    xT [F, n] and splits them evenly across 8 cores (capacity 6656
    tokens/core = 13 blocks of 512; valid count 49152 +- 111, so 6656
    per core is a +37 sigma bound. Any overflow beyond capacity is
    computed on the host — correctness never depends on the bound).
  - Device computes outT = relu(W^T @ xT + b) in transposed space:
      * W [F, U] already has the contraction dim on partitions, so W
        chunks are the stationary operand — NO PE transposes at all.
      * bias lands on partitions (u-chunks), so it rides the ACT
        eviction (activation bias operand) — no bias matmuls.
      * fp16 operands: 1 cyc/row PE rate (same as bf16), half the DMA.
  - Per core: 13 token-blocks of 512, grouped in superblocks of 2048
    so each stationary W chunk serves 4 consecutive matmuls.
    PE work = 832 matmuls x 512 rows ~= 178 us; ACT evicts psum with
    relu+bias; DMA in/out ~28 MB fully overlapped.
"""
import numpy as np
import concourse.bass as bass
import concourse.bacc as bacc
import concourse.mybir as mybir
import concourse.tile as tile
from concourse.bass_utils import run_bass_kernel_spmd

B, V, E, NN = 4, 16384, 4, 32
F, U = 1024, 1024
NCORES = 8
P = 128
BLK = 512                    # tokens per psum bank / matmul free dim
NBLK = 13                    # token-blocks per core (capacity 6656)
CAP = NBLK * BLK             # 6656 tokens per core
SUPERS = [(0, 1024), (1024, 2048), (3072, 2048), (5120, 1536)]
NWARM = 24                   # PE prewarm matmuls issued during DMA fill
CF = F // P                  # 8 contraction chunks
CU = U // P                  # 8 output-partition chunks

f32 = mybir.dt.float32
f16 = mybir.dt.float16
AF = mybir.ActivationFunctionType


def _build():
    nc = bacc.Bacc("TRN2", target_bir_lowering=False, debug=False,
                   num_devices=NCORES)
    xt_d = nc.dram_tensor("xt", [F, CAP], f16, kind="ExternalInput")
    w_d = nc.dram_tensor("weight", [F, U], f16, kind="ExternalInput")
    bias_d = nc.dram_tensor("bias", [P, CU], f32, kind="ExternalInput")
    out_d = nc.dram_tensor("outT", [U, CAP], f16, kind="ExternalOutput")
    scratch_d = nc.dram_tensor("scratch", [P, 1], f16, kind="ExternalOutput")

    with tile.TileContext(nc) as tc:
        with tc.tile_pool(name="const", bufs=1) as const, \
             tc.tile_pool(name="xp", bufs=2) as xp, \
             tc.tile_pool(name="op", bufs=3) as op, \
             tc.tile_pool(name="psp", bufs=2, space="PSUM") as psp:

            # ---- PE prewarm: keep the PE busy from t=0 so the HAM
            # clock-gate is at 8/8 when the real matmuls start; runs
            # while the weight/x DMA fill is in flight. Consumed via a
            # scratch store so DCE keeps it.
            # Consumers stay OFF the critical queues: DVE evicts the warm
            # psum (ACT does the real evictions) and the scratch store
            # goes via the gpsimd DMA queue (sync queue carries the input
            # fill — a blocked sync dma_start would stall the whole fill).
            warm_in = const.tile([P, BLK], f16)
            nc.gpsimd.memset(warm_in[:], 0.0)
            wps = psp.tile([P, BLK], f32, tag="ps")
            for i in range(NWARM):
                nc.tensor.matmul(wps[:], warm_in[:, 0:P], warm_in[:],
                                 start=(i == 0), stop=(i == NWARM - 1))
            warm_out = op.tile([P, 1], f16, tag="warm")
            nc.vector.tensor_copy(out=warm_out[:], in_=wps[:, 0:1])
            nc.gpsimd.dma_start(scratch_d.ap()[:, :], warm_out[:])

            # ---- resident constants: W chunks + bias ----
            # w_sb[:, f*U + j] = W[f*P + p, j]  (slab f = W rows f*P..)
            # Issued from the (otherwise idle at startup) Vector/Scalar
            # queues so the fill builds DMA depth in parallel with the
            # xs0 loads on the Sync queue — one engine alone issues a
            # dma_start only every ~650ns and underfeeds the HW queues.
            w_sb = const.tile([P, CF * U], f16)
            for f in range(CF):
                eng = nc.gpsimd if f % 2 == 0 else nc.scalar
                eng.dma_start(w_sb[:, f * U:(f + 1) * U],
                              w_d.ap()[f * P:(f + 1) * P, :])
            bias_sb = const.tile([P, CU], f32)
            nc.gpsimd.dma_start(bias_sb[:], bias_d.ap())

            for off, W in SUPERS:
                nb = W // BLK
                # xs holds CF f-chunk slabs side by side: chunk f at
                # cols [f*W, (f+1)*W)
                xs = xp.tile([P, CF * W], f16, tag="xs")
                for f in range(CF):
                    nc.sync.dma_start(xs[:, f * W:(f + 1) * W],
                                      xt_d.ap()[f * P:(f + 1) * P,
                                                off:off + W])
                for u in range(CU):
                    ps = psp.tile([P, W], f32, tag="ps")
                    for f in range(CF):
                        lhsT = w_sb[:, f * U + u * P: f * U + (u + 1) * P]
                        for b in range(nb):
                            nc.tensor.matmul(
                                ps[:, b * BLK:(b + 1) * BLK],
                                lhsT,
                                xs[:, f * W + b * BLK: f * W + (b + 1) * BLK],
                                start=(f == 0), stop=(f == CF - 1))
                    r = op.tile([P, W], f16, tag="r")
                    nc.scalar.activation(r[:], ps[:], AF.Relu,
                                         bias=bias_sb[:, u:u + 1])
                    nc.sync.dma_start(out_d.ap()[u * P:(u + 1) * P,
                                                 off:off + W], r[:])

    nc.compile()
    return nc


_nc_cache = None


def _get_nc():
    global _nc_cache
    if _nc_cache is None:
        _nc_cache = _build()
    return _nc_cache


def _preprocess(inputs):
    """Host-side: mask, compaction, fp16 transpose, per-core split."""
    feats2 = np.asarray(inputs["features"], dtype=np.float32).reshape(B * V, F)
    adj2 = np.asarray(inputs["adjacency"]).reshape(B * V, E * NN)
    valid = adj2.max(axis=1) >= 0
    idx = np.flatnonzero(valid)
    dev_idx = idx[:NCORES * CAP]          # device-computed valid tokens
    ovf_idx = idx[NCORES * CAP:]          # host fallback (statistically never)

    w16 = np.ascontiguousarray(inputs["kernel"], dtype=np.float16)
    bias = np.asarray(inputs["bias"], dtype=np.float32).reshape(-1)
    bias_dev = np.ascontiguousarray(bias.reshape(CU, P).T, dtype=np.float32)

    n = dev_idx.size
    counts = [(n + NCORES - 1 - i) // NCORES for i in range(NCORES)]
    starts = np.cumsum([0] + counts)
    in_maps, core_idx = [], []
    for i in range(NCORES):
        ci = dev_idx[starts[i]:starts[i + 1]]
        core_idx.append(ci)
        xti = np.zeros((F, CAP), dtype=np.float16)
        if ci.size:
            xti[:, :ci.size] = feats2[ci].T.astype(np.float16)
        in_maps.append({"xt": xti, "weight": w16, "bias": bias_dev})
    return feats2, valid, core_idx, ovf_idx, in_maps


def _make_in_maps(inputs):
    return _preprocess(inputs)[4]


def kernel(adjacency, features, kernel, bias):
    nc = _get_nc()
    inputs = {"adjacency": adjacency, "features": features,
              "kernel": kernel, "bias": bias}
    feats2, valid, core_idx, ovf_idx, in_maps = _preprocess(inputs)
    res = run_bass_kernel_spmd(nc, in_maps, list(range(NCORES)))

    out = np.empty((B * V, U), dtype=np.float32)
    out[~valid] = feats2[~valid]
    for i in range(NCORES):
        ci = core_idx[i]
        if ci.size:
            oT = res.results[i]["outT"]
            out[ci] = oT[:, :ci.size].T.astype(np.float32)
    if ovf_idx.size:
        w32 = np.asarray(kernel, dtype=np.float32)
        b32 = np.asarray(bias, dtype=np.float32).reshape(-1)
        out[ovf_idx] = np.maximum(feats2[ovf_idx] @ w32 + b32, 0.0)
    return out.reshape(B, V, U)


# revision 14
# speedup vs baseline: 1.1761x; 1.0211x over previous
"""Trainium2 Bass kernel for nn_ConvGraphSelfLoop.

out = where(any(adj>=0, axes -1,-2), relu(features @ W + b), features)

Strategy (device does the GEMM, host does layout + select):
  - A vertex is "valid" iff any adjacency entry >= 0. Invalid vertices
    pass their input features through untouched — the host writes those
    directly from the fp32 input, so the device only transforms valid
    vertices (~75% of 65536).
  - Host compacts the valid vertices, casts to fp16, transposes to
    xT [F, n] and splits them evenly across 8 cores (capacity 6656
    tokens/core = 13 blocks of 512; valid count 49152 +- 111, so 6656
    per core is a +37 sigma bound. Any overflow beyond capacity is
    computed on the host — correctness never depends on the bound).
  - Device computes outT = relu(W^T @ xT + b) in transposed space:
      * W [F, U] already has the contraction dim on partitions, so W
        chunks are the stationary operand — NO PE transposes at all.
      * bias lands on partitions (u-chunks), so it rides the ACT
        eviction (activation bias operand) — no bias matmuls.
      * fp16 operands: 1 cyc/row PE rate (same as bf16), half the DMA.
  - Per core: 13 token-blocks of 512, grouped in superblocks of 2048
    so each stationary W chunk serves 4 consecutive matmuls.
    PE work = 832 matmuls x 512 rows ~= 178 us; ACT evicts psum with
    relu+bias; DMA in/out ~28 MB fully overlapped.
"""
import numpy as np
import concourse.bass as bass
import concourse.bacc as bacc
import concourse.mybir as mybir
import concourse.tile as tile
from concourse.bass_utils import run_bass_kernel_spmd

B, V, E, NN = 4, 16384, 4, 32
F, U = 1024, 1024
NCORES = 8
P = 128
BLK = 512                    # tokens per psum bank / matmul free dim
NBLK = 13                    # token-blocks per core (capacity 6656)
CAP = NBLK * BLK             # 6656 tokens per core
SUPERS = [(0, 1024), (1024, 2048), (3072, 2048), (5120, 1536)]
NWARM = 60                   # PE prewarm matmuls sized to span the DMA fill
CF = F // P                  # 8 contraction chunks
CU = U // P                  # 8 output-partition chunks

f32 = mybir.dt.float32
f16 = mybir.dt.float16
AF = mybir.ActivationFunctionType


def _build():
    nc = bacc.Bacc("TRN2", target_bir_lowering=False, debug=False,
                   num_devices=NCORES)
    xt_d = nc.dram_tensor("xt", [F, CAP], f16, kind="ExternalInput")
    w_d = nc.dram_tensor("weight", [F, U], f16, kind="ExternalInput")
    bias_d = nc.dram_tensor("bias", [P, CU], f32, kind="ExternalInput")
    out_d = nc.dram_tensor("outT", [U, CAP], f16, kind="ExternalOutput")
    scratch_d = nc.dram_tensor("scratch", [P, 1], f16, kind="ExternalOutput")

    with tile.TileContext(nc) as tc:
        with tc.tile_pool(name="const", bufs=1) as const, \
             tc.tile_pool(name="xp", bufs=2) as xp, \
             tc.tile_pool(name="op", bufs=3) as op, \
             tc.tile_pool(name="psp", bufs=2, space="PSUM") as psp:

            # ---- PE prewarm: keep the PE busy from t=0 so the HAM
            # clock-gate is at 8/8 when the real matmuls start; runs
            # while the weight/x DMA fill is in flight. Consumed via a
            # scratch store so DCE keeps it.
            # Consumers stay OFF the critical queues: DVE evicts the warm
            # psum (ACT does the real evictions) and the scratch store
            # goes via the gpsimd DMA queue (sync queue carries the input
            # fill — a blocked sync dma_start would stall the whole fill).
            warm_in = const.tile([P, BLK], f16)
            nc.gpsimd.memset(warm_in[:], 0.0)
            wps = psp.tile([P, BLK], f32, tag="ps")
            for i in range(NWARM):
                nc.tensor.matmul(wps[:], warm_in[:, 0:P], warm_in[:],
                                 start=(i == 0), stop=(i == NWARM - 1))
            warm_out = op.tile([P, 1], f16, tag="warm")
            nc.vector.tensor_copy(out=warm_out[:], in_=wps[:, 0:1])
            nc.gpsimd.dma_start(scratch_d.ap()[:, :], warm_out[:])

            # ---- resident constants: W chunks + bias ----
            # w_sb[:, f*U + j] = W[f*P + p, j]  (slab f = W rows f*P..)
            # Issued from the (otherwise idle at startup) Vector/Scalar
            # queues so the fill builds DMA depth in parallel with the
            # xs0 loads on the Sync queue — one engine alone issues a
            # dma_start only every ~650ns and underfeeds the HW queues.
            w_sb = const.tile([P, CF * U], f16)
            for f in range(CF):
                eng = nc.gpsimd if f % 2 == 0 else nc.scalar
                eng.dma_start(w_sb[:, f * U:(f + 1) * U],
                              w_d.ap()[f * P:(f + 1) * P, :])
            bias_sb = const.tile([P, CU], f32)
            nc.gpsimd.dma_start(bias_sb[:], bias_d.ap())

            for off, W in SUPERS:
                nb = W // BLK
                # xs holds CF f-chunk slabs side by side: chunk f at
                # cols [f*W, (f+1)*W)
                xs = xp.tile([P, CF * W], f16, tag="xs")
                for f in range(CF):
                    nc.sync.dma_start(xs[:, f * W:(f + 1) * W],
                                      xt_d.ap()[f * P:(f + 1) * P,
                                                off:off + W])
                for u in range(CU):
                    ps = psp.tile([P, W], f32, tag="ps")
                    for f in range(CF):
                        lhsT = w_sb[:, f * U + u * P: f * U + (u + 1) * P]
                        for b in range(nb):
                            nc.tensor.matmul(
                                ps[:, b * BLK:(b + 1) * BLK],
                                lhsT,
                                xs[:, f * W + b * BLK: f * W + (b + 1) * BLK],
                                start=(f == 0), stop=(f == CF - 1))
                    r = op.tile([P, W], f16, tag="r")
                    nc.scalar.activation(r[:], ps[:], AF.Relu,
                                         bias=bias_sb[:, u:u + 1])
                    nc.sync.dma_start(out_d.ap()[u * P:(u + 1) * P,
                                                 off:off + W], r[:])

    nc.compile()
    return nc


_nc_cache = None


def _get_nc():
    global _nc_cache
    if _nc_cache is None:
        _nc_cache = _build()
    return _nc_cache


def _preprocess(inputs):
    """Host-side: mask, compaction, fp16 transpose, per-core split."""
    feats2 = np.asarray(inputs["features"], dtype=np.float32).reshape(B * V, F)
    adj2 = np.asarray(inputs["adjacency"]).reshape(B * V, E * NN)
    valid = adj2.max(axis=1) >= 0
    idx = np.flatnonzero(valid)
    dev_idx = idx[:NCORES * CAP]          # device-computed valid tokens
    ovf_idx = idx[NCORES * CAP:]          # host fallback (statistically never)

    w16 = np.ascontiguousarray(inputs["kernel"], dtype=np.float16)
    bias = np.asarray(inputs["bias"], dtype=np.float32).reshape(-1)
    bias_dev = np.ascontiguousarray(bias.reshape(CU, P).T, dtype=np.float32)

    n = dev_idx.size
    counts = [(n + NCORES - 1 - i) // NCORES for i in range(NCORES)]
    starts = np.cumsum([0] + counts)
    in_maps, core_idx = [], []
    for i in range(NCORES):
        ci = dev_idx[starts[i]:starts[i + 1]]
        core_idx.append(ci)
        xti = np.zeros((F, CAP), dtype=np.float16)
        if ci.size:
            xti[:, :ci.size] = feats2[ci].T.astype(np.float16)
        in_maps.append({"xt": xti, "weight": w16, "bias": bias_dev})
    return feats2, valid, core_idx, ovf_idx, in_maps


def _make_in_maps(inputs):
    return _preprocess(inputs)[4]


def kernel(adjacency, features, kernel, bias):
    nc = _get_nc()
    inputs = {"adjacency": adjacency, "features": features,
              "kernel": kernel, "bias": bias}
    feats2, valid, core_idx, ovf_idx, in_maps = _preprocess(inputs)
    res = run_bass_kernel_spmd(nc, in_maps, list(range(NCORES)))

    out = np.empty((B * V, U), dtype=np.float32)
    out[~valid] = feats2[~valid]
    for i in range(NCORES):
        ci = core_idx[i]
        if ci.size:
            oT = res.results[i]["outT"]
            out[ci] = oT[:, :ci.size].T.astype(np.float32)
    if ovf_idx.size:
        w32 = np.asarray(kernel, dtype=np.float32)
        b32 = np.asarray(bias, dtype=np.float32).reshape(-1)
        out[ovf_idx] = np.maximum(feats2[ovf_idx] @ w32 + b32, 0.0)
    return out.reshape(B, V, U)


# revision 17
# speedup vs baseline: 1.2536x; 1.0659x over previous
"""Trainium2 Bass kernel for nn_ConvGraphSelfLoop.

out = where(any(adj>=0, axes -1,-2), relu(features @ W + b), features)

Strategy (device does the GEMM, host does layout + select):
  - A vertex is "valid" iff any adjacency entry >= 0. Invalid vertices
    pass their input features through untouched — the host writes those
    directly from the fp32 input, so the device only transforms valid
    vertices (~75% of 65536).
  - Host compacts the valid vertices, casts to fp16, transposes to
    xT [F, n] and splits them evenly across 8 cores (capacity 6656
    tokens/core = 13 blocks of 512; valid count 49152 +- 111, so 6656
    per core is a +37 sigma bound. Any overflow beyond capacity is
    computed on the host — correctness never depends on the bound).
  - Device computes outT = relu(W^T @ xT + b) in transposed space:
      * W [F, U] already has the contraction dim on partitions, so W
        chunks are the stationary operand — NO PE transposes at all.
      * bias lands on partitions (u-chunks), so it rides the ACT
        eviction (activation bias operand) — no bias matmuls.
      * fp16 operands: 1 cyc/row PE rate (same as bf16), half the DMA.
  - Per core: 13 token-blocks of 512, grouped in superblocks of 2048
    so each stationary W chunk serves 4 consecutive matmuls.
    PE work = 832 matmuls x 512 rows ~= 178 us; ACT evicts psum with
    relu+bias; DMA in/out ~28 MB fully overlapped.
"""
import numpy as np
import concourse.bass as bass
import concourse.bacc as bacc
import concourse.mybir as mybir
import concourse.tile as tile
from concourse.bass_utils import run_bass_kernel_spmd

B, V, E, NN = 4, 16384, 4, 32
F, U = 1024, 1024
NCORES = 8
P = 128
BLK = 512                    # tokens per psum bank / matmul free dim
NBLK = 13                    # token-blocks per core (capacity 6656)
CAP = NBLK * BLK             # 6656 tokens per core
SUPERS = [(0, 2048), (2048, 2048), (4096, 2048), (6144, 512)]
CF = F // P                  # 8 contraction chunks
CU = U // P                  # 8 output-partition chunks

f32 = mybir.dt.float32
f16 = mybir.dt.float16
AF = mybir.ActivationFunctionType


def _build():
    nc = bacc.Bacc("TRN2", target_bir_lowering=False, debug=False,
                   num_devices=NCORES)
    xt_d = nc.dram_tensor("xt", [F, CAP], f16, kind="ExternalInput")
    w_d = nc.dram_tensor("weight", [F, U], f16, kind="ExternalInput")
    bias_d = nc.dram_tensor("bias", [P, CU], f32, kind="ExternalInput")
    out_d = nc.dram_tensor("outT", [U, CAP], f16, kind="ExternalOutput")

    with tile.TileContext(nc) as tc:
        with tc.tile_pool(name="const", bufs=1) as const, \
             tc.tile_pool(name="xp", bufs=2) as xp, \
             tc.tile_pool(name="op", bufs=3) as op, \
             tc.tile_pool(name="psp", bufs=2, space="PSUM") as psp:

            # ---- resident constants: W chunks + bias ----
            # w_sb[:, f*U + j] = W[f*P + p, j]  (slab f = W rows f*P..)
            # Issued from the (otherwise idle at startup) Vector/Scalar
            # queues so the fill builds DMA depth in parallel with the
            # xs0 loads on the Sync queue — one engine alone issues a
            # dma_start only every ~650ns and underfeeds the HW queues.
            w_sb = const.tile([P, CF * U], f16)
            for f in range(CF):
                eng = nc.gpsimd if f % 2 == 0 else nc.scalar
                eng.dma_start(w_sb[:, f * U:(f + 1) * U],
                              w_d.ap()[f * P:(f + 1) * P, :])
            bias_sb = const.tile([P, CU], f32)
            nc.gpsimd.dma_start(bias_sb[:], bias_d.ap())

            for off, W in SUPERS:
                nb = W // BLK
                # xs holds CF f-chunk slabs side by side: chunk f at
                # cols [f*W, (f+1)*W)
                xs = xp.tile([P, CF * W], f16, tag="xs")
                for f in range(CF):
                    nc.sync.dma_start(xs[:, f * W:(f + 1) * W],
                                      xt_d.ap()[f * P:(f + 1) * P,
                                                off:off + W])
                for u in range(CU):
                    ps = psp.tile([P, W], f32, tag="ps")
                    for f in range(CF):
                        lhsT = w_sb[:, f * U + u * P: f * U + (u + 1) * P]
                        for b in range(nb):
                            nc.tensor.matmul(
                                ps[:, b * BLK:(b + 1) * BLK],
                                lhsT,
                                xs[:, f * W + b * BLK: f * W + (b + 1) * BLK],
                                start=(f == 0), stop=(f == CF - 1))
                    r = op.tile([P, W], f16, tag="r")
                    nc.scalar.activation(r[:], ps[:], AF.Relu,
                                         bias=bias_sb[:, u:u + 1])
                    nc.sync.dma_start(out_d.ap()[u * P:(u + 1) * P,
                                                 off:off + W], r[:])

    nc.compile()
    return nc


_nc_cache = None


def _get_nc():
    global _nc_cache
    if _nc_cache is None:
        _nc_cache = _build()
    return _nc_cache


def _preprocess(inputs):
    """Host-side: mask, compaction, fp16 transpose, per-core split."""
    feats2 = np.asarray(inputs["features"], dtype=np.float32).reshape(B * V, F)
    adj2 = np.asarray(inputs["adjacency"]).reshape(B * V, E * NN)
    valid = adj2.max(axis=1) >= 0
    idx = np.flatnonzero(valid)
    dev_idx = idx[:NCORES * CAP]          # device-computed valid tokens
    ovf_idx = idx[NCORES * CAP:]          # host fallback (statistically never)

    w16 = np.ascontiguousarray(inputs["kernel"], dtype=np.float16)
    bias = np.asarray(inputs["bias"], dtype=np.float32).reshape(-1)
    bias_dev = np.ascontiguousarray(bias.reshape(CU, P).T, dtype=np.float32)

    n = dev_idx.size
    counts = [(n + NCORES - 1 - i) // NCORES for i in range(NCORES)]
    starts = np.cumsum([0] + counts)
    in_maps, core_idx = [], []
    for i in range(NCORES):
        ci = dev_idx[starts[i]:starts[i + 1]]
        core_idx.append(ci)
        xti = np.zeros((F, CAP), dtype=np.float16)
        if ci.size:
            xti[:, :ci.size] = feats2[ci].T.astype(np.float16)
        in_maps.append({"xt": xti, "weight": w16, "bias": bias_dev})
    return feats2, valid, core_idx, ovf_idx, in_maps


def _make_in_maps(inputs):
    return _preprocess(inputs)[4]


def kernel(adjacency, features, kernel, bias):
    nc = _get_nc()
    inputs = {"adjacency": adjacency, "features": features,
              "kernel": kernel, "bias": bias}
    feats2, valid, core_idx, ovf_idx, in_maps = _preprocess(inputs)
    res = run_bass_kernel_spmd(nc, in_maps, list(range(NCORES)))

    out = np.empty((B * V, U), dtype=np.float32)
    out[~valid] = feats2[~valid]
    for i in range(NCORES):
        ci = core_idx[i]
        if ci.size:
            oT = res.results[i]["outT"]
            out[ci] = oT[:, :ci.size].T.astype(np.float32)
    if ovf_idx.size:
        w32 = np.asarray(kernel, dtype=np.float32)
        b32 = np.asarray(bias, dtype=np.float32).reshape(-1)
        out[ovf_idx] = np.maximum(feats2[ovf_idx] @ w32 + b32, 0.0)
    return out.reshape(B, V, U)
